# revision 1
# baseline (speedup 1.0000x reference)
"""Trainium2 Bass kernel for a leaky CTRNN (nn_RNN_25451976196554).

Math (per reference):
    alpha = 1/tau; h0c = clip(h0, -1, 1); h broadcast over batch
    per step t: pre = h @ W_hh + u_t @ W_uh + b_h
                h'  = (1-alpha)*h + alpha*tanh(pre)
                y_t = softmax(h' @ W_hy + b_y)

Strategy: data-parallel over batch (8 batch rows per core, 8 cores).
Per core, three phases:
  P1: U_proj = u @ W_uh + b_h for all t (batched matmul, fp32r), -> DRAM.
  P2: sequential recurrence over T=512 steps. Batch-stationary matmuls:
      lhsT = h^T slices [128, 8] (stationary), rhs = W_hh K-tiles streamed
      as the moving operand in fp32r (1 cycle/row at N=512). The new state
      is transposed back to h^T layout with DVE 32x32 stream transposes.
  P3: y = softmax(h_hist @ W_hy + b_y), batched over all (b, t).

Layouts (per core):
  uT   [256, 4096]  : u^T, columns indexed bt = b*T + t (b-major)
  hT   [128, 512]   : h^T packed; chunk k (h dims 128k..128k+128) lives in
                      cols [32k, 32k+8) (cols 32k+8..32k+32 are zero padding
                      written by the 32x32 block transposes).
  Uc   [T, 8, 2048] : U projection, per-step slice contiguous.
  Hh   [T, 8, 2048] : h state history, per-step slice contiguous.
  y    [4096, 256]  : output rows bt = b*T + t (b-major).
"""

import numpy as np

import concourse.bass as bass
import concourse.mybir as mybir
import concourse.tile as tile
from concourse import bacc
from concourse.bass import ds
from concourse.bass_utils import run_bass_kernel_spmd
from concourse.masks import make_identity

N_IN, N_H, N_OUT = 256, 2048, 256
BATCH, T = 64, 512
NCORES = 8
BC = BATCH // NCORES          # 8 batch rows per core
BT = BC * T                   # 4096
KT = N_H // 128               # 16 K tiles over the hidden dim
KIN = N_IN // 128             # 2 K tiles over the input dim
NCH = 4                       # 512-wide psum chunks over N_H
F32 = mybir.dt.float32
F32R = mybir.dt.float32r
AF = mybir.ActivationFunctionType
STAGGERED = True
P2_DMA = True  # timing experiments only; False skips h-store/u-prefetch


def _build(fast: bool, repeat: int = 1, ncores: int = NCORES):
    """repeat: re-run the P2 recurrence `repeat` times inside the NEFF.
    Output is unaffected (each rep restarts from h0); used to time P2
    through the noisy host-side wall clock (slope between repeat counts)."""
    nc = bacc.Bacc("TRN2", target_bir_lowering=False, debug=False,
                   num_devices=ncores)

    uT = nc.declare_dram_parameter("uT", [N_IN, BT], F32, isOutput=False)
    Whh = nc.declare_dram_parameter("Whh", [N_H, N_H], F32, isOutput=False)
    Wuh = nc.declare_dram_parameter("Wuh", [N_IN, N_H], F32, isOutput=False)
    Why = nc.declare_dram_parameter("Why", [N_H, N_OUT], F32, isOutput=False)
    hT0 = nc.declare_dram_parameter("hT0", [128, KT * 32], F32, isOutput=False)
    if not fast:
        alpha_b = nc.declare_dram_parameter("alpha_b", [32, N_H], F32, isOutput=False)
        beta_b = nc.declare_dram_parameter("beta_b", [32, N_H], F32, isOutput=False)
        bh_b = nc.declare_dram_parameter("bh_b", [128, N_H], F32, isOutput=False)
        by_b = nc.declare_dram_parameter("by_b", [128, N_OUT], F32, isOutput=False)
        h0_b = nc.declare_dram_parameter("h0_b", [32, N_H], F32, isOutput=False)
    y = nc.declare_dram_parameter("y", [BT, N_OUT], F32, isOutput=True)

    Uc = nc.dram_tensor("Uc", [T, BC, N_H], F32)
    Hh = nc.dram_tensor("Hh", [T, BC, N_H], F32)
    Uc_v = Uc.ap().rearrange("t b n -> b t n")
    Hh_v = Hh.ap().rearrange("t b n -> b t n")

    with tile.TileContext(nc) as tc:
        with tc.tile_pool(name="persist", bufs=1) as persist, \
             tc.tile_pool(name="stage", bufs=2) as stage:

            # ---- P1: U projection ----
            with tc.tile_pool(name="p1", bufs=3) as p1, \
                 tc.tile_pool(name="psA", bufs=2, space="PSUM") as psA:
                wuh_r = []
                for k in range(KIN):
                    st = stage.tile([128, N_H], F32, tag="wstage")
                    nc.sync.dma_start(out=st[:, :], in_=Wuh[k * 128:(k + 1) * 128, :])
                    wr = p1.tile([128, N_H], F32R, tag=f"wuh{k}", bufs=1)
                    nc.vector.tensor_copy(wr[:, :], st[:, :])
                    wuh_r.append(wr)
                if not fast:
                    bh_sb = p1.tile([128, N_H], F32, tag="bh", bufs=1)
                    nc.sync.dma_start(out=bh_sb[:, :], in_=bh_b[:, :])

                for i in list(range(BT // 128)) * repeat:
                    b = i // 4
                    t0 = (i % 4) * 128
                    uts = []
                    for k in range(KIN):
                        st = p1.tile([128, 128], F32, tag=f"ustage{k}")
                        nc.sync.dma_start(
                            out=st[:, :],
                            in_=uT[k * 128:(k + 1) * 128, i * 128:(i + 1) * 128])
                        ur = p1.tile([128, 128], F32R, tag=f"ur{k}")
                        nc.vector.tensor_copy(ur[:, :], st[:, :])
                        uts.append(ur)
                    acc = p1.tile([128, N_H], F32, tag="uacc")
                    for ch in range(NCH):
                        ps = psA.tile([128, 512], F32, tag=f"ps{ch}")
                        for k in range(KIN):
                            nc.tensor.matmul(
                                ps[:, :], uts[k][:, :],
                                wuh_r[k][:, ch * 512:(ch + 1) * 512],
                                start=(k == 0), stop=(k == KIN - 1))
                        if fast:
                            nc.vector.tensor_copy(
                                acc[:, ch * 512:(ch + 1) * 512], ps[:, :])
                        else:
                            nc.vector.tensor_add(
                                acc[:, ch * 512:(ch + 1) * 512], ps[:, :],
                                bh_sb[:, ch * 512:(ch + 1) * 512])
                    nc.sync.dma_start(
                        out=Uc[t0:t0 + 128, b:b + 1, :], in_=acc[:, :])

            # ---- resident W_hh (rounded to fp32r for the PE) ----
            with tc.tile_pool(name="whhp", bufs=1) as whhp:
                whh_r = []
                for k in range(KT):
                    st = stage.tile([128, N_H], F32, tag="wstage")
                    nc.sync.dma_start(out=st[:, :], in_=Whh[k * 128:(k + 1) * 128, :])
                    wr = whhp.tile([128, N_H], F32R, tag=f"whh{k}")
                    nc.vector.tensor_copy(wr[:, :], st[:, :])
                    whh_r.append(wr)

                # ---- P2: recurrence ----
                hT_a = persist.tile([128, KT * 32], F32R, tag="hTa")
                hT_b = persist.tile([128, KT * 32], F32R, tag="hTb")
                hT_s = persist.tile([128, KT * 32], F32, tag="hTs")
                th = persist.tile([32, N_H], F32, tag="th")
                pre = persist.tile([32, N_H], F32, tag="pre")
                ua = persist.tile([BC, N_H], F32, tag="ua")
                ub = persist.tile([BC, N_H], F32, tag="ub")
                nc.vector.memset(th[0:32, :], 0.0)
                if not fast:
                    alpha_sb = persist.tile([32, N_H], F32, tag="alpha")
                    beta_sb = persist.tile([32, N_H], F32, tag="beta")
                    h_cur = persist.tile([32, N_H], F32, tag="hcur")
                    nc.sync.dma_start(out=alpha_sb[:, :], in_=alpha_b[:, :])
                    nc.sync.dma_start(out=beta_sb[:, :], in_=beta_b[:, :])
                st = stage.tile([128, KT * 32], F32, tag="h0stage")
                nc.sync.dma_start(out=st[:, :], in_=hT0[:, :])

                with tc.tile_pool(name="psB", bufs=2, space="PSUM") as psB:

                    def step(tix, src, dst, u_tile, pf_tix):
                        # per 512-wide chunk: matmul bank -> +u -> tanh ->
                        # 32x32 transposes -> f32r round, so the elementwise
                        # tail of chunk ch pipelines under later banks' PE
                        # work and only the last chunk's tail is exposed.
                        state = th if fast else h_cur
                        sv = state[0:32, :].rearrange("p (i f c) -> p i f c",
                                                      f=4, c=32)
                        for ch in range(NCH):
                            sl = slice(ch * 512, (ch + 1) * 512)
                            ps = psB.tile([BC, 512], F32, tag=f"ps{ch}")
                            for k in range(KT):
                                nc.tensor.matmul(
                                    ps[:, :], src[:, 32 * k:32 * k + BC],
                                    whh_r[k][:, sl],
                                    start=(k == 0), stop=(k == KT - 1))
                            nc.vector.tensor_add(pre[0:BC, sl], ps[:, :],
                                                 u_tile[0:BC, sl])
                            nc.scalar.activation(th[0:BC, sl], pre[0:BC, sl],
                                                 AF.Tanh)
                            if not fast:
                                nc.vector.tensor_mul(th[0:BC, sl],
                                                     th[0:BC, sl],
                                                     alpha_sb[0:BC, sl])
                                nc.vector.tensor_mul(h_cur[0:BC, sl],
                                                     h_cur[0:BC, sl],
                                                     beta_sb[0:BC, sl])
                                nc.vector.tensor_add(h_cur[0:BC, sl],
                                                     h_cur[0:BC, sl],
                                                     th[0:BC, sl])
                            for g in range(4):
                                dv = hT_s[32 * g:32 * (g + 1), :].rearrange(
                                    "p (i c) -> p i c", c=32)
                                nc.vector.transpose(
                                    dv[:, 4 * ch:4 * (ch + 1), :],
                                    sv[:, 4 * ch:4 * (ch + 1), g, :])
                            nc.vector.tensor_copy(
                                dst[:, 128 * ch:128 * (ch + 1)],
                                hT_s[:, 128 * ch:128 * (ch + 1)])
                        # store state, prefetch u
                        if P2_DMA:
                            nc.sync.dma_start(out=Hh_v[:, ds(tix, 1), :],
                                              in_=state[0:BC, :])
                            if pf_tix is not None:
                                nc.sync.dma_start(out=u_tile[0:BC, :],
                                                  in_=Uc_v[:, ds(pf_tix, 1), :])

                    for _rep in range(repeat):
                        nc.vector.tensor_copy(hT_a[:, :], st[:, :])
                        if not fast:
                            nc.sync.dma_start(out=h_cur[:, :], in_=h0_b[:, :])
                        nc.sync.dma_start(out=ua[:, :], in_=Uc_v[:, ds(0, 1), :])
                        nc.sync.dma_start(out=ub[:, :], in_=Uc_v[:, ds(1, 1), :])
                        with tc.For_i(0, (T - 4) // 2, 1,
                                      staggered_reset=STAGGERED) as iv:
                            t0 = iv * 2
                            step(t0, hT_a, hT_b, ua, t0 + 2)
                            step(t0 + 1, hT_b, hT_a, ub, t0 + 3)
                        step(T - 4, hT_a, hT_b, ua, T - 2)
                        step(T - 3, hT_b, hT_a, ub, T - 1)
                        step(T - 2, hT_a, hT_b, ua, None)
                        step(T - 1, hT_b, hT_a, ub, None)

            # ---- P3: output projection + softmax ----
            with tc.tile_pool(name="p3", bufs=3) as p3, \
                 tc.tile_pool(name="psC", bufs=2, space="PSUM") as psC:
                why_r = []
                for k in range(KT):
                    st = stage.tile([128, N_OUT], F32, tag="whystage")
                    nc.sync.dma_start(out=st[:, :],
                                      in_=Why[k * 128:(k + 1) * 128, :])
                    wr = p3.tile([128, N_OUT], F32R, tag=f"why{k}", bufs=1)
                    nc.vector.tensor_copy(wr[:, :], st[:, :])
                    why_r.append(wr)
                ident = p3.tile([128, 128], F32, tag="ident", bufs=1)
                make_identity(nc, ident[:, :])
                if not fast:
                    by_sb = p3.tile([128, N_OUT], F32, tag="by", bufs=1)
                    nc.sync.dma_start(out=by_sb[:, :], in_=by_b[:, :])

                for i in list(range(BT // 128)) * repeat:
                    b = i // 4
                    t0 = (i % 4) * 128
                    htile = p3.tile([128, N_H], F32, tag="h3")
                    nc.sync.dma_start(out=htile[:, :],
                                      in_=Hh[t0:t0 + 128, b:b + 1, :])
                    hT3 = p3.tile([128, N_H], F32R, tag="hT3", bufs=2)
                    psy = psC.tile([128, N_OUT], F32, tag="psy")
                    for k in range(KT):
                        pst = psC.tile([128, 128], F32, tag=f"pst{k % 2}")
                        nc.tensor.transpose(
                            pst[:, :], htile[:, k * 128:(k + 1) * 128],
                            ident[:, :])
                        nc.vector.tensor_copy(
                            hT3[:, k * 128:(k + 1) * 128], pst[:, :])
                    for k in range(KT):
                        nc.tensor.matmul(
                            psy[:, :], hT3[:, k * 128:(k + 1) * 128],
                            why_r[k][:, :], start=(k == 0), stop=(k == KT - 1))
                    logits = p3.tile([128, N_OUT], F32, tag="logits")
                    if fast:
                        nc.vector.tensor_copy(logits[:, :], psy[:, :])
                    else:
                        nc.vector.tensor_add(logits[:, :], psy[:, :],
                                             by_sb[:, :])
                    nm = p3.tile([128, 1], F32, tag="nm")
                    nc.vector.tensor_reduce(nm[:, :], logits[:, :],
                                            axis=mybir.AxisListType.X,
                                            op=mybir.AluOpType.max)
                    nc.vector.tensor_scalar_mul(nm[:, :], nm[:, :], -1.0)
                    e = p3.tile([128, N_OUT], F32, tag="e")
                    s = p3.tile([128, 1], F32, tag="s")
                    nc.scalar.activation(e[:, :], logits[:, :], AF.Exp,
                                         bias=nm[:, :], scale=1.0,
                                         accum_out=s[:, :])
                    r = p3.tile([128, 1], F32, tag="r")
                    nc.vector.reciprocal(r[:, :], s[:, :])
                    yt = p3.tile([128, N_OUT], F32, tag="yt")
                    nc.vector.tensor_scalar_mul(yt[:, :], e[:, :], r[:, :])
                    nc.sync.dma_start(out=y[i * 128:(i + 1) * 128, :],
                                      in_=yt[:, :])

    nc.compile()
    return nc


_NC_CACHE = {}


def _get_nc(fast: bool):
    if fast not in _NC_CACHE:
        _NC_CACHE[fast] = _build(fast)
    return _NC_CACHE[fast]


def kernel(u, W_uh, W_hh, W_hy, b_h, b_y, h0, tau):
    u = np.ascontiguousarray(np.asarray(u, dtype=np.float32))
    W_uh = np.ascontiguousarray(np.asarray(W_uh, dtype=np.float32))
    W_hh = np.ascontiguousarray(np.asarray(W_hh, dtype=np.float32))
    W_hy = np.ascontiguousarray(np.asarray(W_hy, dtype=np.float32))
    b_h = np.asarray(b_h, dtype=np.float32)
    b_y = np.asarray(b_y, dtype=np.float32)
    h0 = np.asarray(h0, dtype=np.float32)
    tau = np.asarray(tau, dtype=np.float32)

    alpha = 1.0 / tau
    fast = bool(np.all(alpha == 1.0) and np.all(b_h == 0.0) and np.all(b_y == 0.0))
    nc = _get_nc(fast)

    h0c = np.clip(h0, -1.0, 1.0)
    hT0 = np.zeros((128, KT * 32), np.float32)
    for k in range(KT):
        hT0[:, 32 * k:32 * k + BC] = h0c[128 * k:128 * (k + 1)][:, None]

    common = {"Whh": W_hh, "Wuh": W_uh, "Why": W_hy, "hT0": hT0}
    if not fast:
        alpha_b = np.zeros((32, N_H), np.float32)
        alpha_b[:BC] = alpha[None, :]
        beta_b = np.zeros((32, N_H), np.float32)
        beta_b[:BC] = (1.0 - alpha)[None, :]
        h0_b = np.zeros((32, N_H), np.float32)
        h0_b[:BC] = h0c[None, :]
        common.update(
            alpha_b=alpha_b, beta_b=beta_b, h0_b=h0_b,
            bh_b=np.ascontiguousarray(np.broadcast_to(b_h[None, :], (128, N_H))),
            by_b=np.ascontiguousarray(np.broadcast_to(b_y[None, :], (128, N_OUT))),
        )

    in_maps = []
    for c in range(NCORES):
        uc = u[c * BC:(c + 1) * BC]                      # [BC, T, N_IN]
        uTc = np.ascontiguousarray(uc.reshape(BC * T, N_IN).T)
        in_maps.append({"uT": uTc, **common})

    res = run_bass_kernel_spmd(nc, in_maps, core_ids=list(range(NCORES)))
    ys = [res.results[c]["y"].reshape(BC, T, N_OUT) for c in range(NCORES)]
    return np.concatenate(ys, axis=0)



# revision 3
# speedup vs baseline: 233.0472x; 233.0472x over previous
"""Trainium2 Bass kernel for a leaky CTRNN (nn_RNN_25451976196554).

Math (per reference):
    alpha = 1/tau; h0c = clip(h0, -1, 1); h broadcast over batch
    per step t: pre = h @ W_hh + u_t @ W_uh + b_h
                h'  = (1-alpha)*h + alpha*tanh(pre)
                y_t = softmax(h' @ W_hy + b_y)

Strategy: data-parallel over batch (8 batch rows per core, 8 cores).
Per core, three phases:
  P1: U_proj = u @ W_uh + b_h for all t (batched matmul, fp32r), -> DRAM.
  P2: sequential recurrence over T=512 steps. Batch-stationary matmuls:
      lhsT = h^T slices [128, 8] (stationary), rhs = W_hh K-tiles streamed
      as the moving operand in fp32r (1 cycle/row at N=512). The new state
      is transposed back to h^T layout with DVE 32x32 stream transposes.
  P3: y = softmax(h_hist @ W_hy + b_y), batched over all (b, t).

Layouts (per core):
  uT   [256, 4096]  : u^T, columns indexed bt = b*T + t (b-major)
  hT   [128, 512]   : h^T packed; chunk k (h dims 128k..128k+128) lives in
                      cols [32k, 32k+8) (cols 32k+8..32k+32 are zero padding
                      written by the 32x32 block transposes).
  Uc   [T, 8, 2048] : U projection, per-step slice contiguous.
  Hh   [T, 8, 2048] : h state history, per-step slice contiguous.
  y    [4096, 256]  : output rows bt = b*T + t (b-major).
"""

import numpy as np

import concourse.bass as bass
import concourse.mybir as mybir
import concourse.tile as tile
from concourse import bacc
from concourse.bass import ds
from concourse.bass_utils import run_bass_kernel_spmd
from concourse.masks import make_identity

N_IN, N_H, N_OUT = 256, 2048, 256
BATCH, T = 64, 512
NCORES = 8
BC = BATCH // NCORES          # 8 batch rows per core
BT = BC * T                   # 4096
KT = N_H // 128               # 16 K tiles over the hidden dim
KIN = N_IN // 128             # 2 K tiles over the input dim
NCH = 4                       # 512-wide psum chunks over N_H
F32 = mybir.dt.float32
F32R = mybir.dt.float32r
BF16 = mybir.dt.bfloat16
AF = mybir.ActivationFunctionType
STAGGERED = True
P2_DMA = True  # timing experiments only; False skips h-store/u-prefetch


def _build(fast: bool, repeat: int = 1, ncores: int = NCORES,
           p2_scale: int = 1, p2_dma: bool = P2_DMA):
    """repeat: re-run the P2 recurrence `repeat` times inside the NEFF.
    Output is unaffected (each rep restarts from h0); used to time P2
    through the noisy host-side wall clock (slope between repeat counts)."""
    nc = bacc.Bacc("TRN2", target_bir_lowering=False, debug=False,
                   num_devices=ncores)

    uT = nc.declare_dram_parameter("uT", [N_IN, BT], F32, isOutput=False)
    Whh = nc.declare_dram_parameter("Whh", [N_H, N_H], F32, isOutput=False)
    Wuh = nc.declare_dram_parameter("Wuh", [N_IN, N_H], F32, isOutput=False)
    Why = nc.declare_dram_parameter("Why", [N_H, N_OUT], F32, isOutput=False)
    hT0 = nc.declare_dram_parameter("hT0", [128, KT * 32], F32, isOutput=False)
    if not fast:
        alpha_b = nc.declare_dram_parameter("alpha_b", [32, N_H], F32, isOutput=False)
        beta_b = nc.declare_dram_parameter("beta_b", [32, N_H], F32, isOutput=False)
        bh_b = nc.declare_dram_parameter("bh_b", [128, N_H], F32, isOutput=False)
        by_b = nc.declare_dram_parameter("by_b", [128, N_OUT], F32, isOutput=False)
        h0_b = nc.declare_dram_parameter("h0_b", [32, N_H], F32, isOutput=False)
    y = nc.declare_dram_parameter("y", [BT, N_OUT], F32, isOutput=True)

    Uc = nc.dram_tensor("Uc", [T, BC, N_H], F32)
    Hh = nc.dram_tensor("Hh", [T, BC, N_H], F32)
    Uc_v = Uc.ap().rearrange("t b n -> b t n")
    Hh_v = Hh.ap().rearrange("t b n -> b t n")

    with tile.TileContext(nc) as tc:
        with tc.tile_pool(name="persist", bufs=1) as persist, \
             tc.tile_pool(name="stage", bufs=2) as stage:

            # ---- P1: U projection ----
            with tc.tile_pool(name="p1", bufs=3) as p1, \
                 tc.tile_pool(name="psA", bufs=2, space="PSUM") as psA:
                wuh_r = []
                for k in range(KIN):
                    st = stage.tile([128, N_H], F32, tag="wstage")
                    nc.sync.dma_start(out=st[:, :], in_=Wuh[k * 128:(k + 1) * 128, :])
                    wr = p1.tile([128, N_H], F32R, tag=f"wuh{k}", bufs=1)
                    nc.vector.tensor_copy(wr[:, :], st[:, :])
                    wuh_r.append(wr)
                if not fast:
                    bh_sb = p1.tile([128, N_H], F32, tag="bh", bufs=1)
                    nc.sync.dma_start(out=bh_sb[:, :], in_=bh_b[:, :])

                for i in list(range(BT // 128)) * repeat:
                    b = i // 4
                    t0 = (i % 4) * 128
                    uts = []
                    for k in range(KIN):
                        st = p1.tile([128, 128], F32, tag=f"ustage{k}")
                        nc.sync.dma_start(
                            out=st[:, :],
                            in_=uT[k * 128:(k + 1) * 128, i * 128:(i + 1) * 128])
                        ur = p1.tile([128, 128], F32R, tag=f"ur{k}")
                        nc.vector.tensor_copy(ur[:, :], st[:, :])
                        uts.append(ur)
                    acc = p1.tile([128, N_H], F32, tag="uacc")
                    for ch in range(NCH):
                        ps = psA.tile([128, 512], F32, tag=f"ps{ch}")
                        for k in range(KIN):
                            nc.tensor.matmul(
                                ps[:, :], uts[k][:, :],
                                wuh_r[k][:, ch * 512:(ch + 1) * 512],
                                start=(k == 0), stop=(k == KIN - 1))
                        if fast:
                            nc.vector.tensor_copy(
                                acc[:, ch * 512:(ch + 1) * 512], ps[:, :])
                        else:
                            nc.vector.tensor_add(
                                acc[:, ch * 512:(ch + 1) * 512], ps[:, :],
                                bh_sb[:, ch * 512:(ch + 1) * 512])
                    nc.sync.dma_start(
                        out=Uc[t0:t0 + 128, b:b + 1, :], in_=acc[:, :])

            # ---- resident W_hh (rounded to fp32r for the PE) ----
            with tc.tile_pool(name="whhp", bufs=1) as whhp:
                whh_r = []
                for k in range(KT):
                    st = stage.tile([128, N_H], F32, tag="wstage")
                    nc.sync.dma_start(out=st[:, :], in_=Whh[k * 128:(k + 1) * 128, :])
                    wr = whhp.tile([128, N_H], BF16, tag=f"whh{k}")
                    nc.vector.tensor_copy(wr[:, :], st[:, :])
                    whh_r.append(wr)

                # ---- P2: recurrence ----
                hT_a = persist.tile([128, KT * 32], BF16, tag="hTa")
                hT_b = persist.tile([128, KT * 32], BF16, tag="hTb")
                hT_s = persist.tile([128, KT * 32], F32, tag="hTs")
                th = persist.tile([32, N_H], F32, tag="th")
                pre = persist.tile([32, N_H], F32, tag="pre")
                ua = persist.tile([BC, N_H], F32, tag="ua")
                ub = persist.tile([BC, N_H], F32, tag="ub")
                nc.vector.memset(th[0:32, :], 0.0)
                if not fast:
                    alpha_sb = persist.tile([32, N_H], F32, tag="alpha")
                    beta_sb = persist.tile([32, N_H], F32, tag="beta")
                    h_cur = persist.tile([32, N_H], F32, tag="hcur")
                    nc.sync.dma_start(out=alpha_sb[:, :], in_=alpha_b[:, :])
                    nc.sync.dma_start(out=beta_sb[:, :], in_=beta_b[:, :])
                st = stage.tile([128, KT * 32], F32, tag="h0stage")
                nc.sync.dma_start(out=st[:, :], in_=hT0[:, :])

                with tc.tile_pool(name="psB", bufs=2, space="PSUM") as psB:

                    def step(tix, src, dst, u_tile, pf_tix):
                        # per 512-wide chunk: matmul bank -> +u -> tanh ->
                        # 32x32 transposes -> f32r round, so the elementwise
                        # tail of chunk ch pipelines under later banks' PE
                        # work and only the last chunk's tail is exposed.
                        state = th if fast else h_cur
                        sv = state[0:32, :].rearrange("p (i f c) -> p i f c",
                                                      f=4, c=32)
                        for ch in range(NCH):
                            sl = slice(ch * 512, (ch + 1) * 512)
                            ps = psB.tile([BC, 512], F32, tag=f"ps{ch}")
                            for k in range(KT):
                                nc.tensor.matmul(
                                    ps[:, :], src[:, 32 * k:32 * k + BC],
                                    whh_r[k][:, sl],
                                    start=(k == 0), stop=(k == KT - 1))
                            nc.vector.tensor_add(pre[0:BC, sl], ps[:, :],
                                                 u_tile[0:BC, sl])
                            nc.scalar.activation(th[0:BC, sl], pre[0:BC, sl],
                                                 AF.Tanh)
                            if not fast:
                                nc.vector.tensor_mul(th[0:BC, sl],
                                                     th[0:BC, sl],
                                                     alpha_sb[0:BC, sl])
                                nc.vector.tensor_mul(h_cur[0:BC, sl],
                                                     h_cur[0:BC, sl],
                                                     beta_sb[0:BC, sl])
                                nc.vector.tensor_add(h_cur[0:BC, sl],
                                                     h_cur[0:BC, sl],
                                                     th[0:BC, sl])
                            for g in range(4):
                                dv = hT_s[32 * g:32 * (g + 1), :].rearrange(
                                    "p (i c) -> p i c", c=32)
                                nc.vector.transpose(
                                    dv[:, 4 * ch:4 * (ch + 1), :],
                                    sv[:, 4 * ch:4 * (ch + 1), g, :])
                            nc.vector.tensor_copy(
                                dst[:, 128 * ch:128 * (ch + 1)],
                                hT_s[:, 128 * ch:128 * (ch + 1)])
                        # store state, prefetch u
                        if p2_dma:
                            nc.sync.dma_start(out=Hh_v[:, ds(tix, 1), :],
                                              in_=state[0:BC, :])
                            if pf_tix is not None:
                                nc.sync.dma_start(out=u_tile[0:BC, :],
                                                  in_=Uc_v[:, ds(pf_tix, 1), :])

                    for _rep in range(repeat):
                        nc.vector.tensor_copy(hT_a[:, :], st[:, :])
                        if not fast:
                            nc.sync.dma_start(out=h_cur[:, :], in_=h0_b[:, :])
                        nc.sync.dma_start(out=ua[:, :], in_=Uc_v[:, ds(0, 1), :])
                        nc.sync.dma_start(out=ub[:, :], in_=Uc_v[:, ds(1, 1), :])
                        with tc.For_i(0, p2_scale * (T - 4) // 2, 1,
                                      staggered_reset=STAGGERED) as iv:
                            t0 = iv * 2
                            step(t0, hT_a, hT_b, ua, t0 + 2)
                            step(t0 + 1, hT_b, hT_a, ub, t0 + 3)
                        step(T - 4, hT_a, hT_b, ua, T - 2)
                        step(T - 3, hT_b, hT_a, ub, T - 1)
                        step(T - 2, hT_a, hT_b, ua, None)
                        step(T - 1, hT_b, hT_a, ub, None)

            # ---- P3: output projection + softmax ----
            with tc.tile_pool(name="p3", bufs=3) as p3, \
                 tc.tile_pool(name="psC", bufs=2, space="PSUM") as psC:
                why_r = []
                for k in range(KT):
                    st = stage.tile([128, N_OUT], F32, tag="whystage")
                    nc.sync.dma_start(out=st[:, :],
                                      in_=Why[k * 128:(k + 1) * 128, :])
                    wr = p3.tile([128, N_OUT], F32R, tag=f"why{k}", bufs=1)
                    nc.vector.tensor_copy(wr[:, :], st[:, :])
                    why_r.append(wr)
                ident = p3.tile([128, 128], F32, tag="ident", bufs=1)
                make_identity(nc, ident[:, :])
                if not fast:
                    by_sb = p3.tile([128, N_OUT], F32, tag="by", bufs=1)
                    nc.sync.dma_start(out=by_sb[:, :], in_=by_b[:, :])

                for i in list(range(BT // 128)) * repeat:
                    b = i // 4
                    t0 = (i % 4) * 128
                    htile = p3.tile([128, N_H], F32, tag="h3")
                    nc.sync.dma_start(out=htile[:, :],
                                      in_=Hh[t0:t0 + 128, b:b + 1, :])
                    hT3 = p3.tile([128, N_H], F32R, tag="hT3", bufs=2)
                    psy = psC.tile([128, N_OUT], F32, tag="psy")
                    for k in range(KT):
                        pst = psC.tile([128, 128], F32, tag=f"pst{k % 2}")
                        nc.tensor.transpose(
                            pst[:, :], htile[:, k * 128:(k + 1) * 128],
                            ident[:, :])
                        nc.vector.tensor_copy(
                            hT3[:, k * 128:(k + 1) * 128], pst[:, :])
                    for k in range(KT):
                        nc.tensor.matmul(
                            psy[:, :], hT3[:, k * 128:(k + 1) * 128],
                            why_r[k][:, :], start=(k == 0), stop=(k == KT - 1))
                    logits = p3.tile([128, N_OUT], F32, tag="logits")
                    if fast:
                        nc.vector.tensor_copy(logits[:, :], psy[:, :])
                    else:
                        nc.vector.tensor_add(logits[:, :], psy[:, :],
                                             by_sb[:, :])
                    nm = p3.tile([128, 1], F32, tag="nm")
                    nc.vector.tensor_reduce(nm[:, :], logits[:, :],
                                            axis=mybir.AxisListType.X,
                                            op=mybir.AluOpType.max)
                    nc.vector.tensor_scalar_mul(nm[:, :], nm[:, :], -1.0)
                    e = p3.tile([128, N_OUT], F32, tag="e")
                    s = p3.tile([128, 1], F32, tag="s")
                    nc.scalar.activation(e[:, :], logits[:, :], AF.Exp,
                                         bias=nm[:, :], scale=1.0,
                                         accum_out=s[:, :])
                    r = p3.tile([128, 1], F32, tag="r")
                    nc.vector.reciprocal(r[:, :], s[:, :])
                    yt = p3.tile([128, N_OUT], F32, tag="yt")
                    nc.vector.tensor_scalar_mul(yt[:, :], e[:, :], r[:, :])
                    nc.sync.dma_start(out=y[i * 128:(i + 1) * 128, :],
                                      in_=yt[:, :])

    nc.compile()
    return nc


_NC_CACHE = {}


def _get_nc(fast: bool):
    if fast not in _NC_CACHE:
        _NC_CACHE[fast] = _build(fast)
    return _NC_CACHE[fast]


def kernel(u, W_uh, W_hh, W_hy, b_h, b_y, h0, tau):
    u = np.ascontiguousarray(np.asarray(u, dtype=np.float32))
    W_uh = np.ascontiguousarray(np.asarray(W_uh, dtype=np.float32))
    W_hh = np.ascontiguousarray(np.asarray(W_hh, dtype=np.float32))
    W_hy = np.ascontiguousarray(np.asarray(W_hy, dtype=np.float32))
    b_h = np.asarray(b_h, dtype=np.float32)
    b_y = np.asarray(b_y, dtype=np.float32)
    h0 = np.asarray(h0, dtype=np.float32)
    tau = np.asarray(tau, dtype=np.float32)

    alpha = 1.0 / tau
    fast = bool(np.all(alpha == 1.0) and np.all(b_h == 0.0) and np.all(b_y == 0.0))
    nc = _get_nc(fast)

    h0c = np.clip(h0, -1.0, 1.0)
    hT0 = np.zeros((128, KT * 32), np.float32)
    for k in range(KT):
        hT0[:, 32 * k:32 * k + BC] = h0c[128 * k:128 * (k + 1)][:, None]

    common = {"Whh": W_hh, "Wuh": W_uh, "Why": W_hy, "hT0": hT0}
    if not fast:
        alpha_b = np.zeros((32, N_H), np.float32)
        alpha_b[:BC] = alpha[None, :]
        beta_b = np.zeros((32, N_H), np.float32)
        beta_b[:BC] = (1.0 - alpha)[None, :]
        h0_b = np.zeros((32, N_H), np.float32)
        h0_b[:BC] = h0c[None, :]
        common.update(
            alpha_b=alpha_b, beta_b=beta_b, h0_b=h0_b,
            bh_b=np.ascontiguousarray(np.broadcast_to(b_h[None, :], (128, N_H))),
            by_b=np.ascontiguousarray(np.broadcast_to(b_y[None, :], (128, N_OUT))),
        )

    in_maps = []
    for c in range(NCORES):
        uc = u[c * BC:(c + 1) * BC]                      # [BC, T, N_IN]
        uTc = np.ascontiguousarray(uc.reshape(BC * T, N_IN).T)
        in_maps.append({"uT": uTc, **common})

    res = run_bass_kernel_spmd(nc, in_maps, core_ids=list(range(NCORES)))
    ys = [res.results[c]["y"].reshape(BC, T, N_OUT) for c in range(NCORES)]
    return np.concatenate(ys, axis=0)

